# revision 6
# baseline (speedup 1.0000x reference)
"""Trainium2 Bass kernel for DTDRLinear: y = hadamard(x) @ (Q*s)^T + bias.

Strategy (8 NeuronCores, data-parallel over tokens, 1024 tok/core):
  - x shard passed transposed (in-major) from host; full Q^T per core.
  - FWHT factorized H_4096 = H_32(outer, on in//128) (x) H_128(inner, in%128):
      * outer 5 butterfly stages on DVE along the free dim (fp16, 2x mode),
      * inner H128 (pre-scaled by 1/64 so the result is the normalized FWHT)
        applied on the PE as a matmul with a constant fp16 matrix.
  - Weights: Q^T int32 slices streamed via HWDGE into staging, cast
    int32->fp16 by gpsimd tensor_copy (Q fits fp16 exactly). 512-wide
    out-feature supers, double-buffered so the next super's weights load a
    full super ahead of use.
  - Main matmul fp16 x fp16 -> f32 PSUM; per-column scale s and bias applied
    at PSUM eviction on DVE against partition-broadcast rows.
  - Phase interleave: chunk c's butterflies+fold feed supers 0/1 main
    matmuls for token tile c, so the PE never waits for the whole FWHT.
"""

import numpy as np

import concourse.bacc as bacc
import concourse.bass as bass
import concourse.mybir as mybir
import concourse.tile as tile
from concourse import bass_utils

TOKENS = 8192
IN = 4096
OUT = 4096
NCORES = 8
T_LOC = TOKENS // NCORES  # 1024

F32 = mybir.dt.float32
F16 = mybir.dt.float16
I32 = mybir.dt.int32

CHUNK = 128               # tokens per butterfly chunk == token tile
NCHUNK = T_LOC // CHUNK   # 8
OSUP = 512                # out-features per weight super
NSUP = OUT // OSUP        # 8
KT = IN // 128            # 32 contraction tiles
ST = 32                   # outer hadamard size (in // 128)

_cache = {}


def _build_nc():
    nc = bacc.Bacc(
        "TRN2",
        target_bir_lowering=False,
        debug=False,
        enable_asserts=False,
        num_devices=NCORES,
        num_swdge_queues=4,
    )
    xT = nc.dram_tensor("xT", [IN, T_LOC], F32, kind="ExternalInput").ap()
    qT = nc.dram_tensor("qT", [IN, OUT], I32, kind="ExternalInput").ap()
    h128 = nc.dram_tensor("h128", [128, 128], F16, kind="ExternalInput").ap()
    s_in = nc.dram_tensor("s_in", [OUT], F32, kind="ExternalInput").ap()
    b_in = nc.dram_tensor("b_in", [OUT], F32, kind="ExternalInput").ap()
    y = nc.dram_tensor("y", [T_LOC, OUT], F32, kind="ExternalOutput").ap()

    xT_v = xT.rearrange("(s p) t -> p s t", p=128)  # [128, 32, T_LOC]
    qT_v = qT.rearrange("(k p) o -> k p o", p=128)  # [32, 128, OUT]

    with tile.TileContext(nc) as tc:
        with (
            tc.tile_pool(name="persist", bufs=1) as persist,
            tc.tile_pool(name="consts", bufs=1) as consts,
            tc.tile_pool(name="fw", bufs=2) as fw,
            tc.tile_pool(name="upool", bufs=2) as upool,
            tc.tile_pool(name="ustage", bufs=4) as ustage,
            tc.tile_pool(name="sbp", bufs=2) as sbp,
            tc.tile_pool(name="ypool", bufs=3) as ypool,
            tc.tile_pool(name="fpsum", bufs=2, space="PSUM") as fpsum,
            tc.tile_pool(name="mpsum", bufs=3, space="PSUM") as mpsum,
        ):
            xhT = persist.tile([128, ST * T_LOC], F16)
            xh_v = xhT.rearrange("p (s t) -> p s t", t=T_LOC)
            h128_sb = consts.tile([128, 128], F16)
            nc.sync.dma_start(out=h128_sb, in_=h128)

            hw_engines = [nc.sync, nc.scalar]

            # pre-issue chunk-0 x load so it isn't queued behind weight loads
            A0 = fw.tile([128, ST * CHUNK], F32, tag="A", name="A0")
            nc.sync.dma_start(out=A0, in_=xT_v[:, :, 0:CHUNK])

            def load_super(sup):
                """scale/bias broadcast rows + dequantized U k-slices."""
                osl = slice(sup * OSUP, (sup + 1) * OSUP)
                s_rep = sbp.tile([128, OSUP], F32, tag="s_rep", name="s_rep")
                b_rep = sbp.tile([128, OSUP], F32, tag="b_rep", name="b_rep")
                nc.gpsimd.dma_start(
                    out=s_rep,
                    in_=bass.AP(
                        tensor=s_in.tensor, offset=sup * OSUP, ap=[[0, 128], [1, OSUP]]
                    ),
                )
                nc.gpsimd.dma_start(
                    out=b_rep,
                    in_=bass.AP(
                        tensor=b_in.tensor, offset=sup * OSUP, ap=[[0, 128], [1, OSUP]]
                    ),
                )
                U = []
                for k in range(KT):
                    us = ustage.tile([128, OSUP], I32, tag="us", name="us")
                    hw_engines[(sup + k) % 2].dma_start(out=us, in_=qT_v[k, :, osl])
                    u = upool.tile([128, OSUP], F16, tag=f"u{k}", name=f"u{k}")
                    nc.gpsimd.tensor_copy(u, us)
                    U.append(u)
                return s_rep, b_rep, U

            def main_tile(sup, t, s_rep, b_rep, U):
                ps = mpsum.tile([128, OSUP], F32, tag="ps", name="ps")
                for k in range(KT):
                    lhs = xhT[:, k * T_LOC + t * CHUNK : k * T_LOC + (t + 1) * CHUNK]
                    nc.tensor.matmul(
                        ps,
                        lhsT=lhs,
                        rhs=U[k],
                        start=(k == 0),
                        stop=(k == KT - 1),
                    )
                ysb = ypool.tile([128, OSUP], F32, tag="ysb", name="ysb")
                nc.vector.tensor_tensor(ysb, ps, s_rep, op=mybir.AluOpType.mult)
                nc.vector.tensor_tensor(ysb, ysb, b_rep, op=mybir.AluOpType.add)
                hw_engines[sup % 2].dma_start(
                    out=y[t * CHUNK : (t + 1) * CHUNK, sup * OSUP : (sup + 1) * OSUP],
                    in_=ysb,
                )

            sb0 = load_super(0)
            sb1 = load_super(1)

            FREE = ST * CHUNK  # 4096 free elements per chunk buffer
            for c in range(NCHUNK):
                if c == 0:
                    A = A0
                else:
                    A = fw.tile([128, FREE], F32, tag="A", name="A")
                    hw_engines[c % 2].dma_start(
                        out=A, in_=xT_v[:, :, c * CHUNK : (c + 1) * CHUNK]
                    )
                B = fw.tile([128, FREE], F16, tag="B", name="B")
                C2 = fw.tile([128, FREE], F16, tag="C2", name="C2", bufs=1)
                order = [B, C2, B, C2, B]
                src = A
                for stage, h in enumerate((1, 2, 4, 8, 16)):
                    dst = order[stage]
                    run = h * CHUNK
                    sv = src.rearrange("p (g two r) -> p g two r", two=2, r=run)
                    dv = dst.rearrange("p (g two r) -> p g two r", two=2, r=run)
                    nc.vector.tensor_add(
                        dv[:, :, 0, :], sv[:, :, 0, :], sv[:, :, 1, :]
                    )
                    nc.vector.tensor_sub(
                        dv[:, :, 1, :], sv[:, :, 0, :], sv[:, :, 1, :]
                    )
                    src = dst
                # inner H128 fold on PE (h128 pre-scaled 1/64); evict to xhT
                SPF = 512 // CHUNK  # s-blocks per 512-wide matmul
                for f in range(FREE // 512):
                    fps = fpsum.tile([128, 512], F32, tag="fps", name="fps")
                    nc.tensor.matmul(
                        fps,
                        lhsT=h128_sb,
                        rhs=src[:, f * 512 : (f + 1) * 512],
                        start=True,
                        stop=True,
                    )
                    nc.scalar.copy(
                        xh_v[:, f * SPF : (f + 1) * SPF, c * CHUNK : (c + 1) * CHUNK],
                        fps,
                    )
                # interleave supers 0/1 main matmuls for this token tile
                main_tile(0, c, *sb0)
                main_tile(1, c, *sb1)

            for sup in range(2, NSUP):
                sbs = load_super(sup)
                for t in range(NCHUNK):
                    main_tile(sup, t, *sbs)

    nc.compile()
    return nc


def _get_nc():
    if "nc" not in _cache:
        _cache["nc"] = _build_nc()
    return _cache["nc"]


def _h128_scaled():
    a = np.eye(128, dtype=np.float64)
    n, hh = 128, 1
    while hh < n:
        a = a.reshape(-1, n // (2 * hh), 2, hh, n)
        l = a[:, :, 0].copy()
        r = a[:, :, 1].copy()
        a[:, :, 0] = l + r
        a[:, :, 1] = l - r
        a = a.reshape(-1, n)
        hh *= 2
    return (a / 64.0).astype(np.float16)


def kernel(x, Q_tilde, s_tilde, bias):
    nc = _get_nc()
    h128 = _h128_scaled()
    qT = np.ascontiguousarray(Q_tilde.T).astype(np.int32)
    s_flat = np.ascontiguousarray(s_tilde.reshape(-1)).astype(np.float32)
    b_flat = np.ascontiguousarray(bias.reshape(-1)).astype(np.float32)
    in_maps = []
    for c in range(NCORES):
        xTc = np.ascontiguousarray(
            x[c * T_LOC : (c + 1) * T_LOC, :].T
        ).astype(np.float32)
        in_maps.append(
            {"xT": xTc, "qT": qT, "h128": h128, "s_in": s_flat, "b_in": b_flat}
        )
    res = bass_utils.run_bass_kernel_spmd(nc, in_maps, list(range(NCORES)))
    yf = np.concatenate([res.results[c]["y"] for c in range(NCORES)], axis=0)
    return yf.astype(np.float32)
